# revision 23
# baseline (speedup 1.0000x reference)
"""AGCA channel-attention forward, data-parallel across 8 TRN2 NeuronCores.

Reference computation (per batch element b):
    y[b,c]   = mean(x[b,c,:,:])                      # global avg pool
    y1[b,h]  = sum_c y[b,c] * W1[h,c]                # 1x1 conv == matmul
    a[b,:]   = softmax(w2 * y1[b,:])                 # over hidden dim
    z[b,k]   = y1[b,k]*a[b,k] + sum_h y1[b,h]*A2[h,k]
    zr       = relu(w3 * z)
    g[b,c]   = sigmoid(sum_h zr[b,h] * W4[c,h])
    out      = x * g[:, :, None, None]

Sharding: pure data parallel on batch (32 -> 4 per core); the tiny params
are replicated. No collectives.

Host-side folding (all inside kernel(), which receives the raw inputs):
  - W1/W4 are pre-transposed to the layouts the TensorEngine wants.
  - the 1/(H*W) of the mean and the w2 scalar fold into the softmax-branch
    copy of W1; sign(w3) folds into the value-branch copy (pushed through
    the linear ops so relu(w3*z) = |w3| * relu(sign(w3)*z)); |w3| folds
    into W4. No runtime scalars reach the device.
  - all params pack into ONE [128, 580] tensor -> one DMA, one funnel copy.

Per-core dataflow: the 4-batch shard (12.85 MB) stays resident in SBUF as
8 blocks of [128 partitions x 3136]; row i = b*256 + c lives at block
k = i // 128 (= 2*b + c//128), partition p = i % 128. Per-row sums via DVE
reduce overlap the 4 block-pair loads; the tiny MLP runs on PE/ACT/DVE;
the per-row sigmoid gate is applied as a per-partition scalar multiply
into per-batch bf16 staging tiles and stored with an f32-casting SWDGE
DMA (bf16 staging halves SBUF and its rounding is ~1e-3 rel, well inside
tolerance).

Hardware note: this neuronxcc build rejects instructions carrying more
than one semaphore wait, so the dataflow keeps every instruction to at
most one new sync domain: matmul operands are funneled through a single
DVE copy of the packed params, PSUM results are bridged to SBUF by
single-input DVE copies before any op that also reads DVE-produced data,
the 4 loads each use a distinct HWDGE semaphore lane while the stores ride
SWDGE lanes, and gate multiplies write fresh staging tiles (never reused,
never in place) so stores wait only on the multiply.
"""

import numpy as np

import concourse.bacc as bacc
import concourse.bass as bass
import concourse.mybir as mybir
import concourse.tile as tile
from concourse.bass_utils import run_bass_kernel_spmd

# Problem shapes (hardcoded: kernel.py must be self-contained).
B, C, H, W = 32, 256, 56, 56
HIDE = 64
NCORES = 8
BL = B // NCORES  # batches per core = 4
HW = H * W  # 3136
ROWS = BL * C  # 1024 rows per core
KBLK = ROWS // 128  # 8 blocks of 128 rows
F32 = mybir.dt.float32
BF16 = mybir.dt.bfloat16
AX = mybir.AxisListType
AF = mybir.ActivationFunctionType

# Packed-parameter column layout: [w2*W1T | s3*W1T | A2 | |w3|*W4T | I4]
PCOLS_W1W2 = 0  # [128, 2*HIDE]
PCOLS_W1S = 2 * HIDE  # [128, 2*HIDE]
PCOLS_A2 = 4 * HIDE  # [64, HIDE]
PCOLS_W4 = 5 * HIDE  # [64, C]
PCOLS_I4 = 5 * HIDE + C  # [4, 4]
PCOLS = PCOLS_I4 + BL  # 580


def _build() -> bass.Bass:
    nc = bacc.Bacc("TRN2", target_bir_lowering=False)
    x_d = nc.dram_tensor("x", [KBLK, 128, HW], F32, kind="ExternalInput")
    params_d = nc.dram_tensor("PARAMS", [128, PCOLS], F32, kind="ExternalInput")
    out_d = nc.dram_tensor("out", [KBLK, 128, HW], F32, kind="ExternalOutput")

    with tile.TileContext(nc) as tc:
        with (
            tc.tile_pool(name="big", bufs=1) as big,
            tc.tile_pool(name="consts", bufs=1) as consts,
            tc.tile_pool(name="small", bufs=1) as small,
            tc.tile_pool(name="psm", bufs=1, space="PSUM") as psm,
            tc.tile_pool(name="psg", bufs=2, space="PSUM") as psg,
        ):
            # ---- params: one DMA + one DVE funnel copy ----
            p_raw = consts.tile([128, PCOLS], F32)
            nc.gpsimd.dma_start(out=p_raw[:, :], in_=params_d[:, :])
            ps = consts.tile([128, PCOLS], F32)
            nc.vector.tensor_copy(out=ps[:, :], in_=p_raw[:, :])

            w1w2 = ps[:, PCOLS_W1W2:PCOLS_W1S].rearrange(
                "p (h d) -> p h d", h=2
            )  # [128, 2, HIDE]
            w1s = ps[:, PCOLS_W1S:PCOLS_A2].rearrange("p (h d) -> p h d", h=2)
            a2s = ps[:HIDE, PCOLS_A2:PCOLS_W4]  # [64, 64]
            w4ts = ps[:HIDE, PCOLS_W4:PCOLS_I4]  # [64, 256]
            i4 = ps[:BL, PCOLS_I4:PCOLS]  # [4, 4]

            # ---- stream x in (one 2-block DMA per batch), per-row sums ----
            xt = big.tile([128, KBLK, HW], F32)
            ysum = small.tile([128, BL, 2], F32)  # ysum[p, b, hf] = row sum
            for b in range(BL):
                nc.sync.dma_start(
                    out=xt[:, 2 * b : 2 * b + 2, :],
                    in_=x_d[2 * b : 2 * b + 2, :, :].rearrange("k p c -> p k c"),
                )
                nc.vector.reduce_sum(
                    out=ysum[:, b, :], in_=xt[:, 2 * b : 2 * b + 2, :], axis=AX.X
                )

            # ---- tiny MLP on the pooled values ----
            # w2*y1 row-major (softmax branch) and sign(w3)*y1 col-major
            # (value branch) straight out of the PE via swapped matmul roles.
            y1wp = psm.tile([BL, HIDE], F32, tag="y1")  # w2*y1 [b, hid]
            y1tp = psm.tile([HIDE, BL], F32, tag="y1t")  # s3*y1T [hid, b]
            for h in range(2):
                nc.tensor.matmul(
                    y1wp[:, :], ysum[:, :, h], w1w2[:, h, :],
                    start=(h == 0), stop=(h == 1),
                )
            for h in range(2):
                nc.tensor.matmul(
                    y1tp[:, :], w1s[:, h, :], ysum[:, :, h],
                    start=(h == 0), stop=(h == 1),
                )

            # softmax over hid (free dim): a = softmax(w2*y1)
            tw2 = small.tile([BL, HIDE], F32)
            nc.vector.tensor_copy(out=tw2[:, :], in_=y1wp[:, :])
            negm = small.tile([BL, 1], F32)
            nc.vector.reduce_max(
                out=negm[:, :], in_=y1wp[:, :], axis=AX.X, negate=True
            )
            y1ts = small.tile([HIDE, BL], F32)
            nc.vector.tensor_copy(out=y1ts[:, :], in_=y1tp[:, :])
            e = small.tile([BL, HIDE], F32)
            nc.scalar.activation(
                out=e[:, :], in_=tw2[:, :], func=AF.Exp, bias=negm[:, :], scale=1.0
            )
            s = small.tile([BL, 1], F32)
            nc.vector.reduce_sum(out=s[:, :], in_=e[:, :], axis=AX.X)
            r = small.tile([BL, 1], F32)
            nc.vector.reciprocal(out=r[:, :], in_=s[:, :])
            a = small.tile([BL, HIDE], F32)
            nc.vector.tensor_scalar_mul(out=a[:, :], in0=e[:, :], scalar1=r[:, :])

            # zT' = s3*y1T * aT + A2^T @ (s3*y1T);  zr = relu(zT')
            atp = psm.tile([HIDE, BL], F32, tag="at")
            nc.tensor.transpose(atp[:, :], a[:, :], i4)
            p3 = psm.tile([HIDE, BL], F32, tag="p3")
            nc.tensor.matmul(p3[:, :], a2s, y1ts[:, :], start=True, stop=True)
            ats = small.tile([HIDE, BL], F32)
            nc.vector.tensor_copy(out=ats[:, :], in_=atp[:, :])
            p3s = small.tile([HIDE, BL], F32)
            nc.vector.tensor_copy(out=p3s[:, :], in_=p3[:, :])
            zt = small.tile([HIDE, BL], F32)
            nc.vector.tensor_mul(out=zt[:, :], in0=y1ts[:, :], in1=ats[:, :])
            nc.vector.tensor_add(out=zt[:, :], in0=zt[:, :], in1=p3s[:, :])
            zr = small.tile([HIDE, BL], F32)
            nc.vector.tensor_scalar_max(out=zr[:, :], in0=zt[:, :], scalar1=0.0)

            # g = sigmoid(|w3| * W4 @ zr) per channel half, laid out to match
            # the x blocks: gt[p, b, hf] gates block k = 2b + hf.
            gt = small.tile([128, BL, 2], F32)
            for h in range(2):
                gp = psg.tile([128, BL], F32, tag="g")
                nc.tensor.matmul(
                    gp[:, :], w4ts[:, h * 128 : (h + 1) * 128], zr[:, :],
                    start=True, stop=True,
                )
                nc.scalar.activation(out=gt[:, :, h], in_=gp[:, :], func=AF.Sigmoid)

            # ---- gate multiply into fresh bf16 staging + casting store ----
            for b in range(BL):
                ot = small.tile([128, 2, HW], BF16, tag=f"ot{b}")
                for hf in range(2):
                    nc.vector.tensor_scalar_mul(
                        out=ot[:, hf, :],
                        in0=xt[:, 2 * b + hf, :],
                        scalar1=gt[:, b, hf : hf + 1],
                    )
                nc.gpsimd.dma_start(
                    out=out_d[2 * b : 2 * b + 2, :, :].rearrange("k p c -> p k c"),
                    in_=ot[:, :, :],
                )

    nc.compile()
    return nc


_CACHE: dict = {}


def _get_nc() -> bass.Bass:
    if "nc" not in _CACHE:
        _CACHE["nc"] = _build()
    return _CACHE["nc"]


def _prep_params(inputs: dict) -> np.ndarray:
    W1 = np.asarray(inputs["W1"], dtype=np.float32)
    W4 = np.asarray(inputs["W4"], dtype=np.float32)
    w2 = float(np.asarray(inputs["w2"], dtype=np.float32)[0])
    w3 = float(np.asarray(inputs["w3"], dtype=np.float32)[0])
    A2 = np.asarray(inputs["A2"], dtype=np.float32)
    assert W1.shape == (HIDE, C) and W4.shape == (C, HIDE)

    # [p, h, hid] layout: W1T[h*128+p, hid] with the channel half h as the
    # middle axis so both halves sit in one contiguous column block.
    base = (W1 / HW).T.reshape(2, 128, HIDE).transpose(1, 0, 2)  # [128, 2, HIDE]
    s3 = float(np.sign(w3))

    params = np.zeros((128, PCOLS), dtype=np.float32)
    params[:, PCOLS_W1W2:PCOLS_W1S] = (w2 * base).reshape(128, 2 * HIDE)
    params[:, PCOLS_W1S:PCOLS_A2] = (s3 * base).reshape(128, 2 * HIDE)
    params[:HIDE, PCOLS_A2:PCOLS_W4] = A2
    params[:HIDE, PCOLS_W4:PCOLS_I4] = abs(w3) * W4.T
    params[:BL, PCOLS_I4:PCOLS] = np.eye(BL, dtype=np.float32)
    return params


def _run(inputs: dict, trace: bool = False):
    x = np.ascontiguousarray(np.asarray(inputs["x"], dtype=np.float32))
    assert x.shape == (B, C, H, W)
    params = _prep_params(inputs)

    in_maps = []
    for i in range(NCORES):
        shard = np.ascontiguousarray(x[i * BL : (i + 1) * BL].reshape(KBLK, 128, HW))
        in_maps.append({"x": shard, "PARAMS": params})

    res = run_bass_kernel_spmd(
        _get_nc(), in_maps, core_ids=list(range(NCORES)), trace=trace
    )
    outs = [r["out"].reshape(BL, C, H, W) for r in res.results]
    return np.concatenate(outs, axis=0), res


def kernel(**inputs) -> np.ndarray:
    out, _ = _run(inputs)
    return out


# revision 24
# speedup vs baseline: 1.1614x; 1.1614x over previous
"""AGCA channel-attention forward, data-parallel across 8 TRN2 NeuronCores.

Reference computation (per batch element b):
    y[b,c]   = mean(x[b,c,:,:])                      # global avg pool
    y1[b,h]  = sum_c y[b,c] * W1[h,c]                # 1x1 conv == matmul
    a[b,:]   = softmax(w2 * y1[b,:])                 # over hidden dim
    z[b,k]   = y1[b,k]*a[b,k] + sum_h y1[b,h]*A2[h,k]
    zr       = relu(w3 * z)
    g[b,c]   = sigmoid(sum_h zr[b,h] * W4[c,h])
    out      = x * g[:, :, None, None]

Sharding: pure data parallel on batch (32 -> 4 per core); the tiny params
are replicated. No collectives.

Host-side folding (all inside kernel(), which receives the raw inputs):
  - W1/W4 are pre-transposed to the layouts the TensorEngine wants.
  - the 1/(H*W) of the mean and the w2 scalar fold into the softmax-branch
    copy of W1; sign(w3) folds into the value-branch copy (pushed through
    the linear ops so relu(w3*z) = |w3| * relu(sign(w3)*z)); |w3| folds
    into W4. No runtime scalars reach the device.
  - all params pack into ONE [128, 580] tensor -> one DMA, one funnel copy.

Per-core dataflow (fully pipelined per batch -- batches are independent
through the whole module): for each of the 4 local batches, one 2-block
load ([128 partitions, 2, 3136], row i = b*256 + c at block k = i//128,
partition i%128), a DVE row-sum, the tiny per-batch MLP on PE/ACT/DVE,
an in-place per-partition-scalar gate multiply, and a store. Loads issue
on the Sync HWDGE ring, stores on the Scalar HWDGE ring, so batch b's
store overlaps batch b+1's load and the HBM read+write streams
interleave at line rate. The shard stays resident in SBUF (12.85 MB).
"""

import numpy as np

import concourse.bacc as bacc
import concourse.bass as bass
import concourse.mybir as mybir
import concourse.tile as tile
from concourse.bass_utils import run_bass_kernel_spmd

# Problem shapes (hardcoded: kernel.py must be self-contained).
B, C, H, W = 32, 256, 56, 56
HIDE = 64
NCORES = 8
BL = B // NCORES  # batches per core = 4
HW = H * W  # 3136
ROWS = BL * C  # 1024 rows per core
KBLK = ROWS // 128  # 8 blocks of 128 rows
F32 = mybir.dt.float32
AX = mybir.AxisListType
AF = mybir.ActivationFunctionType

# Packed-parameter column layout: [w2*W1T | s3*W1T | A2 | |w3|*W4T | I4]
PCOLS_W1W2 = 0  # [128, 2*HIDE]
PCOLS_W1S = 2 * HIDE  # [128, 2*HIDE]
PCOLS_A2 = 4 * HIDE  # [64, HIDE]
PCOLS_W4 = 5 * HIDE  # [64, C]
PCOLS_I4 = 5 * HIDE + C  # [4, 4]
PCOLS = PCOLS_I4 + BL  # 580


def _build() -> bass.Bass:
    nc = bacc.Bacc("TRN2", target_bir_lowering=False)
    x_d = nc.dram_tensor("x", [KBLK, 128, HW], F32, kind="ExternalInput")
    params_d = nc.dram_tensor("PARAMS", [128, PCOLS], F32, kind="ExternalInput")
    out_d = nc.dram_tensor("out", [KBLK, 128, HW], F32, kind="ExternalOutput")

    with tile.TileContext(nc) as tc:
        with (
            tc.tile_pool(name="big", bufs=1) as big,
            tc.tile_pool(name="consts", bufs=1) as consts,
            tc.tile_pool(name="small", bufs=2) as small,
            tc.tile_pool(name="gpool", bufs=1) as gpool,
            tc.tile_pool(name="psm1", bufs=1, space="PSUM") as psm1,
            tc.tile_pool(name="psm2", bufs=2, space="PSUM") as psm2,
            tc.tile_pool(name="psg", bufs=2, space="PSUM") as psg,
        ):
            # ---- params: one DMA + one DVE funnel copy ----
            p_raw = consts.tile([128, PCOLS], F32)
            nc.gpsimd.dma_start(out=p_raw[:, :], in_=params_d[:, :])
            ps = consts.tile([128, PCOLS], F32)
            nc.vector.tensor_copy(out=ps[:, :], in_=p_raw[:, :])

            w1w2 = ps[:, PCOLS_W1W2:PCOLS_W1S].rearrange(
                "p (h d) -> p h d", h=2
            )  # [128, 2, HIDE]
            w1s = ps[:, PCOLS_W1S:PCOLS_A2].rearrange("p (h d) -> p h d", h=2)
            a2s = ps[:HIDE, PCOLS_A2:PCOLS_W4]  # [64, 64]
            w4ts = ps[:HIDE, PCOLS_W4:PCOLS_I4]  # [64, 256]
            i1 = ps[:1, PCOLS_I4 : PCOLS_I4 + 1]  # [1, 1] == 1.0

            xt = big.tile([128, KBLK, HW], F32)
            ysum = gpool.tile([128, BL, 2], F32)  # ysum[p, b, hf] = row sum
            gt = gpool.tile([128, BL, 2], F32)  # gt[p, b, hf] gates blk 2b+hf

            for b in range(BL):
                # load both blocks of batch b; per-row spatial sums
                nc.sync.dma_start(
                    out=xt[:, 2 * b : 2 * b + 2, :],
                    in_=x_d[2 * b : 2 * b + 2, :, :].rearrange("k p c -> p k c"),
                )
                nc.vector.reduce_sum(
                    out=ysum[:, b, :], in_=xt[:, 2 * b : 2 * b + 2, :], axis=AX.X
                )

                # w2*y1 row-major (softmax branch) and sign(w3)*y1 col-major
                # (value branch) straight off the PE via swapped matmul roles.
                y1wp = psm2.tile([1, HIDE], F32, tag="y1")
                y1tp = psm1.tile([HIDE, 1], F32, tag="y1t")
                for h in range(2):
                    nc.tensor.matmul(
                        y1wp[:, :], ysum[:, b, h : h + 1], w1w2[:, h, :],
                        start=(h == 0), stop=(h == 1),
                    )
                for h in range(2):
                    nc.tensor.matmul(
                        y1tp[:, :], w1s[:, h, :], ysum[:, b, h : h + 1],
                        start=(h == 0), stop=(h == 1),
                    )

                # a = softmax(w2*y1) over hid (free dim)
                tw2 = small.tile([1, HIDE], F32, tag="tw2")
                nc.vector.tensor_copy(out=tw2[:, :], in_=y1wp[:, :])
                negm = small.tile([1, 1], F32, tag="negm")
                nc.vector.reduce_max(
                    out=negm[:, :], in_=y1wp[:, :], axis=AX.X, negate=True
                )
                y1ts = small.tile([HIDE, 1], F32, tag="y1ts")
                nc.vector.tensor_copy(out=y1ts[:, :], in_=y1tp[:, :])
                e = small.tile([1, HIDE], F32, tag="e")
                nc.scalar.activation(
                    out=e[:, :], in_=tw2[:, :], func=AF.Exp,
                    bias=negm[:, :], scale=1.0,
                )
                s = small.tile([1, 1], F32, tag="s")
                nc.vector.reduce_sum(out=s[:, :], in_=e[:, :], axis=AX.X)
                r = small.tile([1, 1], F32, tag="r")
                nc.vector.reciprocal(out=r[:, :], in_=s[:, :])
                a = small.tile([1, HIDE], F32, tag="a")
                nc.vector.tensor_scalar_mul(out=a[:, :], in0=e[:, :], scalar1=r[:, :])

                # zT' = s3*y1T * aT + A2^T @ (s3*y1T);  zr = relu(zT')
                atp = psm1.tile([HIDE, 1], F32, tag="at")
                nc.tensor.transpose(atp[:, :], a[:, :], i1)
                p3 = psm1.tile([HIDE, 1], F32, tag="p3")
                nc.tensor.matmul(p3[:, :], a2s, y1ts[:, :], start=True, stop=True)
                ats = small.tile([HIDE, 1], F32, tag="ats")
                nc.vector.tensor_copy(out=ats[:, :], in_=atp[:, :])
                p3s = small.tile([HIDE, 1], F32, tag="p3s")
                nc.vector.tensor_copy(out=p3s[:, :], in_=p3[:, :])
                zt = small.tile([HIDE, 1], F32, tag="zt")
                nc.vector.tensor_mul(out=zt[:, :], in0=y1ts[:, :], in1=ats[:, :])
                nc.vector.tensor_add(out=zt[:, :], in0=zt[:, :], in1=p3s[:, :])
                zr = small.tile([HIDE, 1], F32, tag="zr")
                nc.vector.tensor_scalar_max(out=zr[:, :], in0=zt[:, :], scalar1=0.0)

                # g = sigmoid(|w3| * W4 @ zr) per channel half
                for h in range(2):
                    gp = psg.tile([128, 1], F32, tag="g")
                    nc.tensor.matmul(
                        gp[:, :], w4ts[:, h * 128 : (h + 1) * 128], zr[:, :],
                        start=True, stop=True,
                    )
                    nc.scalar.activation(
                        out=gt[:, b, h : h + 1], in_=gp[:, :], func=AF.Sigmoid
                    )

                # in-place gate multiply, then store batch b (Scalar HWDGE
                # ring, so stores overlap later batches' Sync-ring loads)
                for hf in range(2):
                    nc.vector.tensor_scalar_mul(
                        out=xt[:, 2 * b + hf, :],
                        in0=xt[:, 2 * b + hf, :],
                        scalar1=gt[:, b, hf : hf + 1],
                    )
                nc.scalar.dma_start(
                    out=out_d[2 * b : 2 * b + 2, :, :].rearrange("k p c -> p k c"),
                    in_=xt[:, 2 * b : 2 * b + 2, :],
                )

    nc.compile()
    return nc


_CACHE: dict = {}


def _get_nc() -> bass.Bass:
    if "nc" not in _CACHE:
        _CACHE["nc"] = _build()
    return _CACHE["nc"]


def _prep_params(inputs: dict) -> np.ndarray:
    W1 = np.asarray(inputs["W1"], dtype=np.float32)
    W4 = np.asarray(inputs["W4"], dtype=np.float32)
    w2 = float(np.asarray(inputs["w2"], dtype=np.float32)[0])
    w3 = float(np.asarray(inputs["w3"], dtype=np.float32)[0])
    A2 = np.asarray(inputs["A2"], dtype=np.float32)
    assert W1.shape == (HIDE, C) and W4.shape == (C, HIDE)

    # [p, h, hid] layout: W1T[h*128+p, hid] with the channel half h as the
    # middle axis so both halves sit in one contiguous column block.
    base = (W1 / HW).T.reshape(2, 128, HIDE).transpose(1, 0, 2)  # [128, 2, HIDE]
    s3 = float(np.sign(w3))

    params = np.zeros((128, PCOLS), dtype=np.float32)
    params[:, PCOLS_W1W2:PCOLS_W1S] = (w2 * base).reshape(128, 2 * HIDE)
    params[:, PCOLS_W1S:PCOLS_A2] = (s3 * base).reshape(128, 2 * HIDE)
    params[:HIDE, PCOLS_A2:PCOLS_W4] = A2
    params[:HIDE, PCOLS_W4:PCOLS_I4] = abs(w3) * W4.T
    params[:BL, PCOLS_I4:PCOLS] = np.eye(BL, dtype=np.float32)
    return params


def _run(inputs: dict, trace: bool = False):
    x = np.ascontiguousarray(np.asarray(inputs["x"], dtype=np.float32))
    assert x.shape == (B, C, H, W)
    params = _prep_params(inputs)

    in_maps = []
    for i in range(NCORES):
        shard = np.ascontiguousarray(x[i * BL : (i + 1) * BL].reshape(KBLK, 128, HW))
        in_maps.append({"x": shard, "PARAMS": params})

    res = run_bass_kernel_spmd(
        _get_nc(), in_maps, core_ids=list(range(NCORES)), trace=trace
    )
    outs = [r["out"].reshape(BL, C, H, W) for r in res.results]
    return np.concatenate(outs, axis=0), res


def kernel(**inputs) -> np.ndarray:
    out, _ = _run(inputs)
    return out


# revision 26
# speedup vs baseline: 1.2171x; 1.0480x over previous
"""AGCA channel-attention forward, data-parallel across 8 TRN2 NeuronCores.

Reference computation (per batch element b):
    y[b,c]   = mean(x[b,c,:,:])                      # global avg pool
    y1[b,h]  = sum_c y[b,c] * W1[h,c]                # 1x1 conv == matmul
    a[b,:]   = softmax(w2 * y1[b,:])                 # over hidden dim
    z[b,k]   = y1[b,k]*a[b,k] + sum_h y1[b,h]*A2[h,k]
    zr       = relu(w3 * z)
    g[b,c]   = sigmoid(sum_h zr[b,h] * W4[c,h])
    out      = x * g[:, :, None, None]

Sharding: pure data parallel on batch (32 -> 4 per core); the tiny params
are replicated. No collectives.

Host-side folding (all inside kernel(), which receives the raw inputs):
  - W1/W4 are pre-transposed to the layouts the TensorEngine wants.
  - the 1/(H*W) of the mean and the w2 scalar fold into the softmax-branch
    copy of W1; sign(w3) folds into the value-branch copy (pushed through
    the linear ops so relu(w3*z) = |w3| * relu(sign(w3)*z)); |w3| folds
    into W4. No runtime scalars reach the device.
  - all params pack into ONE [128, 580] tensor -> one DMA, one funnel copy.

Per-core dataflow (fully pipelined per batch -- batches are independent
through the whole module): for each of the 4 local batches, one 2-block
load ([128 partitions, 2, 3136], row i = b*256 + c at block k = i//128,
partition i%128), a DVE row-sum, the tiny per-batch MLP on PE/ACT/DVE,
an in-place per-partition-scalar gate multiply, and a store. Loads issue
on the Sync HWDGE ring, stores on the Scalar HWDGE ring, so batch b's
store overlaps batch b+1's load and the HBM read+write streams
interleave at line rate. The shard stays resident in SBUF (12.85 MB).
"""

import numpy as np

import concourse.bacc as bacc
import concourse.bass as bass
import concourse.mybir as mybir
import concourse.tile as tile
from concourse.bass_utils import run_bass_kernel_spmd

# Problem shapes (hardcoded: kernel.py must be self-contained).
B, C, H, W = 32, 256, 56, 56
HIDE = 64
NCORES = 8
BL = B // NCORES  # batches per core = 4
HW = H * W  # 3136
ROWS = BL * C  # 1024 rows per core
KBLK = ROWS // 128  # 8 blocks of 128 rows
F32 = mybir.dt.float32
AX = mybir.AxisListType
AF = mybir.ActivationFunctionType

# Packed-parameter column layout: [w2*W1T | s3*W1T | A2 | |w3|*W4T | I4]
PCOLS_W1W2 = 0  # [128, 2*HIDE]
PCOLS_W1S = 2 * HIDE  # [128, 2*HIDE]
PCOLS_A2 = 4 * HIDE  # [64, HIDE]
PCOLS_W4 = 5 * HIDE  # [64, C]
PCOLS_I4 = 5 * HIDE + C  # [4, 4]
PCOLS = PCOLS_I4 + BL  # 580


def _build() -> bass.Bass:
    nc = bacc.Bacc("TRN2", target_bir_lowering=False)
    x_d = nc.dram_tensor("x", [KBLK, 128, HW], F32, kind="ExternalInput")
    params_d = nc.dram_tensor("PARAMS", [128, PCOLS], F32, kind="ExternalInput")
    out_d = nc.dram_tensor("out", [KBLK, 128, HW], F32, kind="ExternalOutput")

    with tile.TileContext(nc) as tc:
        with (
            tc.tile_pool(name="big", bufs=1) as big,
            tc.tile_pool(name="consts", bufs=1) as consts,
            tc.tile_pool(name="small", bufs=2) as small,
            tc.tile_pool(name="gpool", bufs=1) as gpool,
            tc.tile_pool(name="psm1", bufs=1, space="PSUM") as psm1,
            tc.tile_pool(name="psm2", bufs=2, space="PSUM") as psm2,
            tc.tile_pool(name="psg", bufs=2, space="PSUM") as psg,
        ):
            # ---- params: one DMA + one DVE funnel copy ----
            p_raw = consts.tile([128, PCOLS], F32)
            nc.gpsimd.dma_start(out=p_raw[:, :], in_=params_d[:, :])
            ps = consts.tile([128, PCOLS], F32)
            nc.vector.tensor_copy(out=ps[:, :], in_=p_raw[:, :])

            w1w2 = ps[:, PCOLS_W1W2:PCOLS_W1S].rearrange(
                "p (h d) -> p h d", h=2
            )  # [128, 2, HIDE]
            w1s = ps[:, PCOLS_W1S:PCOLS_A2].rearrange("p (h d) -> p h d", h=2)
            a2s = ps[:HIDE, PCOLS_A2:PCOLS_W4]  # [64, 64]
            w4ts = ps[:HIDE, PCOLS_W4:PCOLS_I4]  # [64, 256]
            i1 = ps[:1, PCOLS_I4 : PCOLS_I4 + 1]  # [1, 1] == 1.0

            xt = big.tile([128, KBLK, HW], F32)
            ysum = gpool.tile([128, BL, 2], F32)  # ysum[p, b, hf] = row sum
            gt = gpool.tile([128, BL, 2], F32)  # gt[p, b, hf] gates blk 2b+hf
            nc.vector.memset(ysum[:, :, :], 0.0)  # ACT accum-copies add into it

            # all loads issue upfront on the Sync HWDGE ring (no waits)
            for b in range(BL):
                nc.sync.dma_start(
                    out=xt[:, 2 * b : 2 * b + 2, :],
                    in_=x_d[2 * b : 2 * b + 2, :, :].rearrange("k p c -> p k c"),
                )

            for b in range(BL):
                # per-row spatial sums: block hf=0 on DVE, block hf=1 on ACT
                # (identity copy with free-dim accumulate) -- in parallel.
                nc.vector.reduce_sum(
                    out=ysum[:, b, 0:1], in_=xt[:, 2 * b, :], axis=AX.X
                )
                nc.scalar.activation(
                    out=xt[:, 2 * b + 1, :],
                    in_=xt[:, 2 * b + 1, :],
                    func=AF.Copy,
                    accum_out=ysum[:, b, 1:2],
                )

                # w2*y1 row-major (softmax branch) and sign(w3)*y1 col-major
                # (value branch) straight off the PE via swapped matmul roles.
                y1wp = psm2.tile([1, HIDE], F32, tag="y1")
                y1tp = psm1.tile([HIDE, 1], F32, tag="y1t")
                for h in range(2):
                    nc.tensor.matmul(
                        y1wp[:, :], ysum[:, b, h : h + 1], w1w2[:, h, :],
                        start=(h == 0), stop=(h == 1),
                    )
                for h in range(2):
                    nc.tensor.matmul(
                        y1tp[:, :], w1s[:, h, :], ysum[:, b, h : h + 1],
                        start=(h == 0), stop=(h == 1),
                    )

                # a = softmax(w2*y1) over hid (free dim)
                tw2 = small.tile([1, HIDE], F32, tag="tw2")
                nc.vector.tensor_copy(out=tw2[:, :], in_=y1wp[:, :])
                negm = small.tile([1, 1], F32, tag="negm")
                nc.vector.reduce_max(
                    out=negm[:, :], in_=y1wp[:, :], axis=AX.X, negate=True
                )
                y1ts = small.tile([HIDE, 1], F32, tag="y1ts")
                nc.vector.tensor_copy(out=y1ts[:, :], in_=y1tp[:, :])
                e = small.tile([1, HIDE], F32, tag="e")
                nc.scalar.activation(
                    out=e[:, :], in_=tw2[:, :], func=AF.Exp,
                    bias=negm[:, :], scale=1.0,
                )
                s = small.tile([1, 1], F32, tag="s")
                nc.vector.reduce_sum(out=s[:, :], in_=e[:, :], axis=AX.X)
                r = small.tile([1, 1], F32, tag="r")
                nc.vector.reciprocal(out=r[:, :], in_=s[:, :])
                a = small.tile([1, HIDE], F32, tag="a")
                nc.vector.tensor_scalar_mul(out=a[:, :], in0=e[:, :], scalar1=r[:, :])

                # zT' = s3*y1T * aT + A2^T @ (s3*y1T);  zr = relu(zT')
                atp = psm1.tile([HIDE, 1], F32, tag="at")
                nc.tensor.transpose(atp[:, :], a[:, :], i1)
                p3 = psm1.tile([HIDE, 1], F32, tag="p3")
                nc.tensor.matmul(p3[:, :], a2s, y1ts[:, :], start=True, stop=True)
                ats = small.tile([HIDE, 1], F32, tag="ats")
                nc.vector.tensor_copy(out=ats[:, :], in_=atp[:, :])
                p3s = small.tile([HIDE, 1], F32, tag="p3s")
                nc.vector.tensor_copy(out=p3s[:, :], in_=p3[:, :])
                zt = small.tile([HIDE, 1], F32, tag="zt")
                nc.vector.tensor_mul(out=zt[:, :], in0=y1ts[:, :], in1=ats[:, :])
                nc.vector.tensor_add(out=zt[:, :], in0=zt[:, :], in1=p3s[:, :])
                zr = small.tile([HIDE, 1], F32, tag="zr")
                nc.vector.tensor_scalar_max(out=zr[:, :], in0=zt[:, :], scalar1=0.0)

                # g = sigmoid(|w3| * W4 @ zr) per channel half
                for h in range(2):
                    gp = psg.tile([128, 1], F32, tag="g")
                    nc.tensor.matmul(
                        gp[:, :], w4ts[:, h * 128 : (h + 1) * 128], zr[:, :],
                        start=True, stop=True,
                    )
                    nc.scalar.activation(
                        out=gt[:, b, h : h + 1], in_=gp[:, :], func=AF.Sigmoid
                    )

                # in-place gate multiply: block hf=0 on DVE, hf=1 on ACT.
                nc.vector.tensor_scalar_mul(
                    out=xt[:, 2 * b, :],
                    in0=xt[:, 2 * b, :],
                    scalar1=gt[:, b, 0:1],
                )
                nc.scalar.mul(
                    out=xt[:, 2 * b + 1, :],
                    in_=xt[:, 2 * b + 1, :],
                    mul=gt[:, b, 1:2],
                )
                # stores alternate between the two HWDGE rings so writes
                # overlap the remaining Sync-ring loads.
                store_engine = nc.scalar if b % 2 == 0 else nc.sync
                store_engine.dma_start(
                    out=out_d[2 * b : 2 * b + 2, :, :].rearrange("k p c -> p k c"),
                    in_=xt[:, 2 * b : 2 * b + 2, :],
                )

    nc.compile()
    return nc


_CACHE: dict = {}


def _get_nc() -> bass.Bass:
    if "nc" not in _CACHE:
        _CACHE["nc"] = _build()
    return _CACHE["nc"]


def _prep_params(inputs: dict) -> np.ndarray:
    W1 = np.asarray(inputs["W1"], dtype=np.float32)
    W4 = np.asarray(inputs["W4"], dtype=np.float32)
    w2 = float(np.asarray(inputs["w2"], dtype=np.float32)[0])
    w3 = float(np.asarray(inputs["w3"], dtype=np.float32)[0])
    A2 = np.asarray(inputs["A2"], dtype=np.float32)
    assert W1.shape == (HIDE, C) and W4.shape == (C, HIDE)

    # [p, h, hid] layout: W1T[h*128+p, hid] with the channel half h as the
    # middle axis so both halves sit in one contiguous column block.
    base = (W1 / HW).T.reshape(2, 128, HIDE).transpose(1, 0, 2)  # [128, 2, HIDE]
    s3 = float(np.sign(w3))

    params = np.zeros((128, PCOLS), dtype=np.float32)
    params[:, PCOLS_W1W2:PCOLS_W1S] = (w2 * base).reshape(128, 2 * HIDE)
    params[:, PCOLS_W1S:PCOLS_A2] = (s3 * base).reshape(128, 2 * HIDE)
    params[:HIDE, PCOLS_A2:PCOLS_W4] = A2
    params[:HIDE, PCOLS_W4:PCOLS_I4] = abs(w3) * W4.T
    params[:BL, PCOLS_I4:PCOLS] = np.eye(BL, dtype=np.float32)
    return params


def _run(inputs: dict, trace: bool = False):
    x = np.ascontiguousarray(np.asarray(inputs["x"], dtype=np.float32))
    assert x.shape == (B, C, H, W)
    params = _prep_params(inputs)

    in_maps = []
    for i in range(NCORES):
        shard = np.ascontiguousarray(x[i * BL : (i + 1) * BL].reshape(KBLK, 128, HW))
        in_maps.append({"x": shard, "PARAMS": params})

    res = run_bass_kernel_spmd(
        _get_nc(), in_maps, core_ids=list(range(NCORES)), trace=trace
    )
    outs = [r["out"].reshape(BL, C, H, W) for r in res.results]
    return np.concatenate(outs, axis=0), res


def kernel(**inputs) -> np.ndarray:
    out, _ = _run(inputs)
    return out
